# revision 24
# baseline (speedup 1.0000x reference)
"""Distributed Trainium2 (8 NeuronCores) kernel for a pre-LN transformer block.

Reference computation (B=2, T=2048, E=1024, H=16, D=64):
    h1 = LN(data); q,k,v = per-head projections; causal attention (scale E^-0.5);
    x = data + concat @ Wfc + bfc; out = x + relu(LN(x) @ W1 + b1) @ W2 + b2

Sharding (Ulysses-style, SPMD-uniform across the 8 cores):
  - rows (b,t) are sharded: core c owns rows [256c, 256c+256) of each batch
    (512 rows/core, held transposed as [E, 512], col order [b0|b1])
  - LN1 + all-head QKV projections computed on local rows in fp8 DoubleRow
    (weights host-prescaled x32; scales folded into the softmax exp and the
    Wfc epilogue), then three combined-batch AllToAlls (Q/V/K) move to head
    sharding; a tiny AllToAll fires at kernel start to absorb collective
    warmup/launch skew off the critical path
  - heads sharded: core c owns heads {2c, 2c+1}; full-T causal attention;
    scores stay transposed [keys, q]; 2 heads pack the 64-deep contraction
    via tile_position row groups; softmax denominators come free from a
    ones-column in V; exp probabilities are written fp8 and the P@V matmul
    runs fp8 DoubleRow over key-chunk pairs
  - per-batch AllToAlls return attention output (fp8, x32) to row sharding
  - Wfc (fp8 DoubleRow) + residual + LN2 + FFN (bf16: fp8 fails the accuracy
    budget here) + residual computed on local rows, pipelined by batch half;
    W2 accumulates e-major with all weights prefetched so outputs drain
    progressively instead of in a serial tail
All collective payloads are fp8. LN stats and softmax exp in f32.
"""
import numpy as np
import ml_dtypes

import concourse.bass as bass
import concourse.bacc as bacc
import concourse.tile as tile
from concourse import mybir
from concourse import bass_utils

FP32 = mybir.dt.float32
BF16 = mybir.dt.bfloat16
FP8 = mybir.dt.float8e4
AF = mybir.ActivationFunctionType
OP = mybir.AluOpType
DR = mybir.MatmulPerfMode.DoubleRow

B, T, E, H, D = 2, 2048, 1024, 16, 64
NC = 8
RPB = T // NC            # 256 rows per batch per core
ROWS = B * RPB           # 512 local rows
NE = E // 128            # 8 tiles over E
NEP = NE // 2            # 4 DoubleRow pairs over E
F4 = 4 * E
NF = F4 // 128           # 32 tiles over 4E
NKT = T // 128           # 16 key tiles per batch
EPS = 1e-5
WS = 32.0                # host prescale on Wq/Wk/Wv/Wfc (folded back downstream)
SC2 = (float(E) ** -0.5) / (WS * WS)   # exp scale with q,k both carrying x32
RG = [list(range(NC))]
USE_DR = True

_last_result = None  # BassKernelResults from the most recent run (for test harness)


def _layernorm(nc, tc, workp, statsp, eps1, x_tiles, g_col, b_col, out_factory,
               psname, c0=0, ncols=ROWS):
    """LayerNorm over the E (partition) axis of 8 [128, *] f32 tiles,
    restricted to columns [c0, c0+ncols). bf16 column sums on PE; f32 stats;
    per-row scale/shift broadcast via K=1 matmuls."""
    ones128 = workp.tile([128, 1], BF16, name=f"{psname}_ones128",
                         tag="lno", bufs=2)
    nc.vector.memset(ones128[:], 1.0)
    cs = slice(c0, c0 + ncols)

    with tc.tile_pool(name=psname, bufs=1, space="PSUM") as ps:
        sum_ps = ps.tile([1, ncols], FP32, name=f"{psname}_sum", tag="sum")
        ssq_ps = ps.tile([1, ncols], FP32, name=f"{psname}_ssq", tag="ssq")
        for e in range(NE):
            xb = workp.tile([128, ncols], BF16, name=f"{psname}_xb{e}",
                            tag="lnsrc", bufs=2)
            nc.vector.tensor_copy(xb[:], x_tiles[e][:, cs])
            sq = workp.tile([128, ncols], BF16, name=f"{psname}_sq{e}",
                            tag="lnsq", bufs=2)
            nc.scalar.activation(sq[:], x_tiles[e][:, cs], AF.Square)
            nc.tensor.matmul(sum_ps[:], ones128[:], xb[:],
                             start=(e == 0), stop=(e == NE - 1))
            nc.tensor.matmul(ssq_ps[:], ones128[:], sq[:],
                             start=(e == 0), stop=(e == NE - 1))
        mean = statsp.tile([1, ncols], FP32, name=f"{psname}_mean", tag="v0")
        nc.vector.tensor_scalar_mul(mean[:], sum_ps[:], 1.0 / E)
        msq = statsp.tile([1, ncols], FP32, name=f"{psname}_msq", tag="v1")
        nc.vector.tensor_mul(msq[:], mean[:], mean[:])
        var = statsp.tile([1, ncols], FP32, name=f"{psname}_var", tag="v2")
        nc.vector.scalar_tensor_tensor(var[:], ssq_ps[:], 1.0 / E, msq[:],
                                       OP.mult, OP.subtract)
        std = statsp.tile([1, ncols], FP32, name=f"{psname}_std", tag="v3")
        nc.scalar.activation(std[:], var[:], AF.Sqrt, bias=eps1[:, 0:1])
        rstd = statsp.tile([1, ncols], FP32, name=f"{psname}_rstd", tag="v4")
        nc.vector.reciprocal(rstd[:], std[:])
        nmrn = statsp.tile([1, ncols], FP32, name=f"{psname}_nmrn", tag="v5")
        nc.vector.scalar_tensor_tensor(nmrn[:], mean[:], -1.0, rstd[:],
                                       OP.mult, OP.mult)
        # broadcast rstd / -mean*rstd across partitions on the (idle) GpSimd
        # engine: keeps them out of PSUM, so the next phase's matmul banks
        # don't WAR-wait on the LN epilogue
        bA = workp.tile([128, ncols], FP32, name=f"{psname}_bA",
                        tag="lnbA", bufs=2)
        nc.gpsimd.partition_broadcast(bA[:], rstd[:])
        bB = workp.tile([128, ncols], FP32, name=f"{psname}_bB",
                        tag="lnbB", bufs=2)
        nc.gpsimd.partition_broadcast(bB[:], nmrn[:])
        for e in range(NE):
            t1 = workp.tile([128, ncols], FP32, name=f"{psname}_t1_{e}",
                            tag="lnt1", bufs=2)
            nc.vector.tensor_mul(t1[:], x_tiles[e][:, cs], bA[:])
            t2 = workp.tile([128, ncols], FP32, name=f"{psname}_t2_{e}",
                            tag="lnt2", bufs=2)
            nc.vector.tensor_add(t2[:], t1[:], bB[:])
            o = out_factory(e)
            nc.scalar.activation(o[:, cs], t2[:], AF.Identity,
                                 bias=b_col(e), scale=g_col(e))


def _build():
    nc = bacc.Bacc("TRN2", target_bir_lowering=False, debug=False, num_devices=NC)

    dataT_d = nc.dram_tensor("dataT", [E, ROWS], FP32, kind="ExternalInput")
    wq_d = nc.dram_tensor("wq", [E, H * D], FP8, kind="ExternalInput")
    wk_d = nc.dram_tensor("wk", [E, H * D], FP8, kind="ExternalInput")
    wv_d = nc.dram_tensor("wv", [E, H * D], FP8, kind="ExternalInput")
    wfc_d = nc.dram_tensor("wfc", [H * D, E], FP8, kind="ExternalInput")
    w1_d = nc.dram_tensor("w1l", [F4, E], BF16, kind="ExternalInput")
    w2_d = nc.dram_tensor("w2", [F4, E], BF16, kind="ExternalInput")
    mask_d = nc.dram_tensor("mask", [128, 128], BF16, kind="ExternalInput")
    g1_d = nc.dram_tensor("g1", [E], FP32, kind="ExternalInput")
    be1_d = nc.dram_tensor("be1", [E], FP32, kind="ExternalInput")
    g2_d = nc.dram_tensor("g2", [E], FP32, kind="ExternalInput")
    be2_d = nc.dram_tensor("be2", [E], FP32, kind="ExternalInput")
    bfc_d = nc.dram_tensor("bfc", [E], FP32, kind="ExternalInput")
    b1_d = nc.dram_tensor("b1", [F4], FP32, kind="ExternalInput")
    b2_d = nc.dram_tensor("b2", [E], FP32, kind="ExternalInput")
    out_d = nc.dram_tensor("outT", [E, ROWS], FP32, kind="ExternalOutput")

    def mm_pairs(ps, lhs_pair, rhs_pair, c, clast):
        """One DoubleRow (or two fallback) matmuls accumulating pair c."""
        if USE_DR:
            nc.tensor.matmul(ps, lhs_pair, rhs_pair,
                             start=(c == 0), stop=(c == clast),
                             perf_mode=DR)
        else:
            for sl in range(2):
                nc.tensor.matmul(ps, lhs_pair[:, sl], rhs_pair[:, sl],
                                 start=(c == 0 and sl == 0),
                                 stop=(c == clast and sl == 1))

    with tile.TileContext(nc) as tc:
        with (
            tc.tile_pool(name="constp", bufs=1) as constp,
            tc.tile_pool(name="datap", bufs=1) as datap,
            tc.tile_pool(name="workp", bufs=4) as workp,
            tc.tile_pool(name="statsp", bufs=1) as statsp,
            tc.tile_pool(name="xhp", bufs=1) as xhp,
            tc.tile_pool(name="w1p", bufs=1) as w1p,
            tc.tile_pool(name="dramp", bufs=1, space="DRAM") as dramp,
        ):
            # skew/warmup absorber: a tiny collective nothing depends on,
            # fired before any compute so the first real A2A finds the CC
            # ring warm and the cores synced
            dumb_in = dramp.tile([NC * 128, 4], FP8, name="dumb_in", tag="di")
            dumb_out = dramp.tile([NC * 128, 4], FP8, name="dumb_out", tag="do")
            zed = constp.tile([128, NC * 4], FP8, name="zed", tag="zed")
            nc.vector.memset(zed[:], 0.0)
            nc.sync.dma_start(
                out=dumb_in[:, :].rearrange("(s p) x -> p s x", p=128),
                in_=zed[:, :].rearrange("p (s x) -> p s x", s=NC))
            nc.gpsimd.collective_compute(
                "AllToAll", OP.bypass, replica_groups=RG,
                ins=[dumb_in[:, :].opt()], outs=[dumb_out[:, :].opt()])

            # ---------- constant / input loads ----------
            mask_sb = constp.tile([128, 128], BF16, name="mask_sb", tag="mask")
            nc.sync.dma_start(out=mask_sb[:], in_=mask_d[:, :])
            eps1 = constp.tile([1, 1], FP32, name="eps1", tag="eps1")
            nc.vector.memset(eps1[:], EPS)
            # preload the sqrt activation table while the data DMAs stream,
            # keeping the ~2.7us ACT_TABLE_LOAD out of LN1's critical chain
            sqw0 = workp.tile([1, 1], FP32, name="sqw0", tag="warm", bufs=2)
            nc.scalar.activation(sqw0[:], eps1[:, 0:1], AF.Sqrt)
            vecs = {}
            for nm, dd, w in (("g1", g1_d, NE), ("be1", be1_d, NE), ("g2", g2_d, NE),
                              ("be2", be2_d, NE), ("bfc", bfc_d, NE), ("b2", b2_d, NE),
                              ("b1", b1_d, NF)):
                t = constp.tile([128, w], FP32, name=f"{nm}_sb", tag=nm)
                nc.sync.dma_start(out=t[:], in_=dd.ap().rearrange("(a b) -> b a", b=128))
                vecs[nm] = t

            data_t = []
            for e in range(NE):
                dt_ = datap.tile([128, ROWS], FP32, name=f"data{e}", tag=f"data{e}")
                nc.sync.dma_start(out=dt_[:], in_=dataT_d[128 * e:128 * (e + 1), :])
                data_t.append(dt_)

            # DRAM bounce buffers for the collectives (all fp8).
            # Q and K ride one A2A: block ft = [Q 128 | K 128] x 512 rows.
            qkb_in = dramp.tile([NC * 256, ROWS], FP8, name="qkb_in", tag="qkb_in")
            qkb_out = dramp.tile([NC * 256, ROWS], FP8, name="qkb_out",
                                 tag="qkb_out")
            vb_in = [dramp.tile([NC * 256, 128], FP8, name=f"vb_in{b}",
                                tag=f"vb_in{b}") for b in range(B)]
            vb_out = [dramp.tile([NC * 256, 128], FP8, name=f"vb_out{b}",
                                 tag=f"vb_out{b}") for b in range(B)]
            a2a_in = [dramp.tile([NC * 128, RPB], FP8, name=f"a2a_in{b}",
                                 tag=f"a2a_in{b}") for b in range(B)]
            a2a_out = [dramp.tile([NC * 128, RPB], FP8, name=f"a2a_out{b}",
                                  tag=f"a2a_out{b}") for b in range(B)]

            with (
                tc.tile_pool(name="wfcp", bufs=1) as wfcp,
                tc.tile_pool(name="ccp", bufs=1) as ccp,
            ):
                # cc_all[p, s, b*256+x] = concat (x32 fp8) for hd-block s
                cc_all = ccp.tile([128, NE, ROWS], FP8, name="cc_all", tag="cc")
                with (
                    tc.tile_pool(name="qtp", bufs=1) as qtp,
                    tc.tile_pool(name="vp", bufs=1) as vp,
                    tc.tile_pool(name="clp", bufs=1) as clp,
                ):
                    QTb = [qtp.tile([128, T], FP8, name=f"QT{b}", tag=f"QT{b}")
                           for b in range(B)]
                    KTb = [qtp.tile([128, T], FP8, name=f"KT{b}", tag=f"KT{b}")
                           for b in range(B)]
                    # v_ab[p, k, 96*h + x]; x=64 is the ones column
                    v_ab = [vp.tile([128, NKT, 192], FP8, name=f"v_all{b}",
                                    tag=f"v_all{b}") for b in range(B)]
                    for b in range(B):
                        nc.vector.memset(
                            v_ab[b][:, :, :].rearrange(
                                "p k (h x) -> p k h x", h=2)[:, :, :, 64:65],
                            1.0)
                    concatL = clp.tile([128, B * T], FP8, name="concatL",
                                       tag="concatL")

                    # --- LN1, fp8 DoubleRow QKV, combined-batch A2As ---
                    with (
                        tc.tile_pool(name="h1lp", bufs=1) as h1lp,
                        tc.tile_pool(name="wqkvp", bufs=1) as wqkvp,
                        tc.tile_pool(name="qklp", bufs=1) as qklp,
                    ):
                        h1p = [h1lp.tile([128, 2, ROWS], FP8, name=f"h1p{c}",
                                         tag=f"h1p{c}") for c in range(NEP)]
                        wq_t, wk_t, wv_t = [], [], []
                        for nm, dd, lst in (("wq", wq_d, wq_t), ("wk", wk_d, wk_t),
                                            ("wv", wv_d, wv_t)):
                            for c in range(NEP):
                                t = wqkvp.tile([128, 2, H * D], FP8,
                                               name=f"{nm}p{c}", tag=f"wqkv{c}",
                                               bufs=2)
                                nc.sync.dma_start(
                                    out=t[:, :, :],
                                    in_=dd[256 * c:256 * (c + 1), :].rearrange(
                                        "(two p) f -> p two f", two=2))
                                lst.append(t)

                        _layernorm(nc, tc, workp, statsp, eps1, data_t,
                                   lambda e: vecs["g1"][:, e:e + 1],
                                   lambda e: vecs["be1"][:, e:e + 1],
                                   lambda e: h1p[e // 2][:, e % 2, :], "ln1")

                        with tc.tile_pool(name="psqkv", bufs=1,
                                          space="PSUM") as psqkv:
                            # Q then K projections -> one combined A2A
                            for nm, wt, half in (("q", wq_t, 0), ("k", wk_t, 1)):
                                pss = [psqkv.tile([128, ROWS], FP32,
                                                  name=f"ps{nm}{i}", tag=f"mm{i}",
                                                  bufs=1) for i in range(NE)]
                                for c in range(NEP):
                                    for ft in range(NE):
                                        mm_pairs(pss[ft][:],
                                                 wt[c][:, :, 128 * ft:128 * (ft + 1)],
                                                 h1p[c][:, :, :], c, NEP - 1)
                                for ft in range(NE):
                                    lt = qklp.tile([128, ROWS], FP8,
                                                   name=f"l{nm}{ft}", tag="qklq",
                                                   bufs=3)
                                    nc.vector.tensor_copy(lt[:], pss[ft][:])
                                    nc.sync.dma_start(
                                        out=qkb_in[256 * ft + 128 * half:
                                                   256 * ft + 128 * (half + 1), :],
                                        in_=lt[:])
                            nc.gpsimd.collective_compute(
                                "AllToAll", OP.bypass, replica_groups=RG,
                                ins=[qkb_in[:, :].opt()],
                                outs=[qkb_out[:, :].opt()])
                            expw = workp.tile([1, 1], FP32, name="expw",
                                              tag="warm", bufs=2)
                            nc.scalar.activation(expw[:], eps1[:, 0:1], AF.Exp)

                            # V projection (rows on partitions), batch-0 chains
                            # first so its A2A fires before batch-1's; each
                            # batch's exchange lands just before its attention
                            pss = [psqkv.tile([128, 512], FP32,
                                              name=f"psv{i}", tag=f"mm{i}",
                                              bufs=1) for i in range(NE)]
                            for bb in range(B):
                                for i in range(NE):
                                    g, rt = divmod(i, 4)
                                    if rt // 2 != bb:
                                        continue
                                    for c in range(NEP):
                                        mm_pairs(pss[i][:],
                                                 h1p[c][:, :, 128 * rt:128 * (rt + 1)],
                                                 wv_t[c][:, :, 512 * g:512 * (g + 1)],
                                                 c, NEP - 1)
                                    jj = rt % 2
                                    lv = qklp.tile([128, 512], FP8,
                                                   name=f"lv{i}", tag="qklq",
                                                   bufs=3)
                                    nc.vector.tensor_copy(lv[:], pss[i][:])
                                    nc.sync.dma_start(
                                        out=vb_in[bb][:, :].rearrange(
                                            "(ft j p) x -> p ft j x",
                                            ft=NE, j=2)[:, 4 * g:4 * g + 4, jj, :],
                                        in_=lv[:, :].rearrange(
                                            "p (f x) -> p f x", f=4))
                                nc.gpsimd.collective_compute(
                                    "AllToAll", OP.bypass, replica_groups=RG,
                                    ins=[vb_in[bb][:, :].opt()],
                                    outs=[vb_out[bb][:, :].opt()])

                        # readbacks, one DMA each, in collective order QK, V
                        for b in range(B):
                            for dst, half in ((QTb, 0), (KTb, 1)):
                                nc.scalar.dma_start(
                                    out=dst[b][:, :].rearrange(
                                        "p (s x) -> p s x", s=NC),
                                    in_=qkb_out[:, :].rearrange(
                                        "(s w p) (b x) -> p w b s x",
                                        w=2, p=128, b=B)[:, half, b, :, :])
                        for b in range(B):
                            for hi in range(2):
                                nc.scalar.dma_start(
                                    out=v_ab[b][:, :, :].rearrange(
                                        "p k (h x) -> p k h x",
                                        h=2)[:, :, hi, 0:64],
                                    in_=vb_out[b][:, :].rearrange(
                                        "(k p) (h x) -> p k h x",
                                        p=128, h=2)[:, :, hi, :])

                    # prefetch Wfc (fp8 pairs) while attention runs
                    wfc_t = []
                    for s in range(NEP):
                        t = wfcp.tile([128, 2, E], FP8, name=f"wfcp{s}",
                                      tag=f"wfc{s}")
                        nc.sync.dma_start(
                            out=t[:, :, :],
                            in_=wfc_d[256 * s:256 * (s + 1), :].rearrange(
                                "(two p) e -> p two e", two=2))
                        wfc_t.append(t)

                    # ------- causal attention for 2 heads, DR P@V -------
                    with (
                        tc.tile_pool(name="pst", bufs=2, space="PSUM") as pst,
                        tc.tile_pool(name="pot", bufs=4, space="PSUM") as pot,
                    ):
                        for b in range(B):
                            for qc in range(T // 512):
                                q0 = 512 * qc
                                nk = 4 * qc + 4
                                ots = []
                                for hi in range(2):
                                    ots.append(pot.tile([65, 512], FP32,
                                                        name=f"ot{b}_{qc}_{hi}",
                                                        tag="ot"))
                                def emit_av(pk, poff, ppexp, last):
                                    for hi in range(2):
                                        nc.tensor.matmul(
                                            ots[hi][:, poff:512],
                                            v_ab[b][:, pk, 96 * hi:96 * hi + 65],
                                            ppexp[:, 512 * hi + poff:
                                                  512 * hi + 512],
                                            start=(pk == 0), stop=last)

                                # software pipeline: scores(k+1) is emitted
                                # before P@V(k), so the PE fills each chunk's
                                # exp latency with the next chunk's scores
                                pend = None
                                for k in range(nk):
                                    off = max(0, 128 * k - q0)
                                    st = pst.tile([128, 1024], FP32,
                                                  name=f"st{b}_{qc}_{k}",
                                                  tag="st")
                                    pexp = workp.tile(
                                        [128, 1024], FP8,
                                        name=f"pex{b}_{qc}_{k}",
                                        tag="pexp", bufs=4)
                                    for hi in range(2):
                                        hp = slice(64 * hi, 64 * (hi + 1))
                                        nc.tensor.matmul(
                                            st[:, 512 * hi + off:512 * hi + 512],
                                            KTb[b][hp, 128 * k:128 * (k + 1)],
                                            QTb[b][hp, q0 + off:q0 + 512],
                                            start=True, stop=True,
                                            tile_position=(64 * hi, 0))
                                    nc.scalar.activation(
                                        pexp[:, :].rearrange(
                                            "p (h x) -> p h x", h=2)[:, :, off:512],
                                        st[:, :].rearrange(
                                            "p (h x) -> p h x", h=2)[:, :, off:512],
                                        AF.Exp, scale=SC2)
                                    if k >= 4 * qc:  # diagonal: causal mask
                                        for hi in range(2):
                                            nc.vector.tensor_mul(
                                                pexp[:, 512 * hi + off:
                                                     512 * hi + off + 128],
                                                pexp[:, 512 * hi + off:
                                                     512 * hi + off + 128],
                                                mask_sb[:])
                                    if pend is not None:
                                        emit_av(*pend, False)
                                    pend = (k, off, pexp)
                                emit_av(*pend, True)
                                for hi in range(2):
                                    rc = statsp.tile([1, 512], FP32,
                                                     name=f"rc{b}_{qc}_{hi}",
                                                     tag="rc", bufs=2)
                                    nc.vector.reciprocal(rc[:], ots[hi][64:65, :])
                                    rbs = workp.tile([64, 512], FP32,
                                                     name=f"rbs{b}_{qc}_{hi}",
                                                     tag="rbs", bufs=3)
                                    nc.gpsimd.partition_broadcast(rbs[:], rc[:])
                                    nc.vector.tensor_mul(
                                        concatL[64 * hi:64 * (hi + 1),
                                                b * T + q0: b * T + q0 + 512],
                                        ots[hi][0:64, :], rbs[:])
                            # batch-b attention done: AllToAll it back to row
                            # sharding while the next batch computes
                            nc.sync.dma_start(
                                out=a2a_in[b][:, :].rearrange(
                                    "(j p) x -> p j x", p=128),
                                in_=concatL[:, b * T:(b + 1) * T].rearrange(
                                    "p (j x) -> p j x", j=NC))
                            nc.gpsimd.collective_compute(
                                "AllToAll", OP.bypass, replica_groups=RG,
                                ins=[a2a_in[b][:, :].opt()],
                                outs=[a2a_out[b][:, :].opt()])
                            nc.scalar.dma_start(
                                out=cc_all[:, :, RPB * b:RPB * (b + 1)],
                                in_=a2a_out[b][:, :].rearrange(
                                    "(s p) x -> p s x", p=128))

                # ---------- batch-half pipelined tail ----------
                x_t, h2_t = [], []
                for e in range(NE):
                    x_t.append(xhp.tile([128, ROWS], BF16, name=f"x{e}",
                                        tag=f"x{e}"))
                    h2_t.append(xhp.tile([128, ROWS], BF16, name=f"h2_{e}",
                                         tag=f"h2{e}"))

                sqw = workp.tile([1, 1], FP32, name="sqw", tag="warm", bufs=2)
                nc.scalar.activation(sqw[:], eps1[:, 0:1], AF.Sqrt)

                def wfc_half(half, ps_pool):
                    for e in range(NE):
                        ps = ps_pool.tile([128, RPB], FP32,
                                          name=f"psx{half}_{e}", tag="mm")
                        for s in range(NEP):
                            mm_pairs(ps[:],
                                     wfc_t[s][:, :, 128 * e:128 * (e + 1)],
                                     cc_all[:, 2 * s:2 * s + 2,
                                            RPB * half:RPB * (half + 1)],
                                     s, NEP - 1)
                        xw = workp.tile([128, RPB], FP32,
                                        name=f"xw{half}_{e}", tag="xw", bufs=3)
                        nc.scalar.activation(xw[:], ps[:], AF.Identity,
                                             bias=vecs["bfc"][:, e:e + 1],
                                             scale=1.0 / (WS * WS))
                        nc.vector.tensor_add(
                            x_t[e][:, RPB * half:RPB * (half + 1)], xw[:],
                            data_t[e][:, RPB * half:RPB * (half + 1)])

                NSPLIT = 16  # zT f-tiles emitted per-half to cover A2A#1

                with (
                    tc.tile_pool(name="rtp", bufs=1) as rtp,
                    tc.tile_pool(name="w2p", bufs=1) as w2p,
                ):
                    r_t = []
                    for f in range(NF):
                        r_t.append(rtp.tile([128, ROWS], BF16, name=f"r{f}",
                                            tag=f"r{f}"))

                    def w1_load(f, tag="w1f", bufs=4):
                        w1f = w1p.tile([128, E], BF16, name=f"w1f{f}", tag=tag,
                                       bufs=bufs)
                        nc.sync.dma_start(out=w1f[:],
                                          in_=w1_d[128 * f:128 * (f + 1), :])
                        return w1f

                    def zt_chain(f, w1f, psz, c0, ncols):
                        ps = psz.tile([128, ncols], FP32,
                                      name=f"psz{f}_{c0}", tag="mm")
                        for e in range(NE):
                            nc.tensor.matmul(
                                ps[:], w1f[:, 128 * e:128 * (e + 1)],
                                h2_t[e][:, c0:c0 + ncols],
                                start=(e == 0), stop=(e == NE - 1))
                        nc.scalar.activation(r_t[f][:, c0:c0 + ncols], ps[:],
                                             AF.Relu, bias=vecs["b1"][:, f:f + 1])

                    with tc.tile_pool(name="psfc", bufs=2, space="PSUM") as psfc:
                        wfc_half(0, psfc)
                    _layernorm(nc, tc, workp, statsp, eps1, x_t,
                               lambda e: vecs["g2"][:, e:e + 1],
                               lambda e: vecs["be2"][:, e:e + 1],
                               lambda e: h2_t[e], "ln2a",
                               c0=0, ncols=RPB)
                    w1fs = {}
                    w2_t = []
                    with tc.tile_pool(name="psz", bufs=2, space="PSUM") as psz:
                        for f in range(NSPLIT):
                            w1fs[f] = w1_load(f, tag=f"w1k{f}", bufs=1)
                            zt_chain(f, w1fs[f], psz, 0, RPB)
                        # batch-1 catch-up (waits on the concat AllToAll)
                        with tc.tile_pool(name="psfc2", bufs=2,
                                          space="PSUM") as psfc2:
                            wfc_half(1, psfc2)
                        _layernorm(nc, tc, workp, statsp, eps1, x_t,
                                   lambda e: vecs["g2"][:, e:e + 1],
                                   lambda e: vecs["be2"][:, e:e + 1],
                                   lambda e: h2_t[e], "ln2b",
                                   c0=RPB, ncols=RPB)
                        for f in range(NSPLIT):
                            zt_chain(f, w1fs[f], psz, RPB, RPB)
                        for f in range(NSPLIT, NF):
                            w1f = w1_load(f)
                            zt_chain(f, w1f, psz, 0, ROWS)
                        # prefetch first-half W2 tiles; the stream overlaps
                        # the tail of the W1 compute
                        NH = NF // 2
                        for f in range(NH):
                            w2t = w2p.tile([128, E], BF16, name=f"w2t{f}",
                                           tag=f"w2_{f % NH}")
                            nc.sync.dma_start(
                                out=w2t[:], in_=w2_d[128 * f:128 * (f + 1), :])
                            w2_t.append(w2t)
                    # two-pass W2 with carried PSUM accumulation (halves the
                    # resident weight footprint); outputs drain per-e in the
                    # second pass instead of in a serial tail
                    with tc.tile_pool(name="psff", bufs=1, space="PSUM") as psff:
                        ff_ps = [psff.tile([128, ROWS], FP32, name=f"ff{e}",
                                           tag=f"ff{e}") for e in range(NE)]
                        for e in range(NE):
                            for f in range(NH):
                                nc.tensor.matmul(ff_ps[e][:],
                                                 w2_t[f][:, 128 * e:128 * (e + 1)],
                                                 r_t[f][:],
                                                 start=(f == 0), stop=False)
                        for f in range(NH, NF):
                            w2t = w2p.tile([128, E], BF16, name=f"w2t{f}",
                                           tag=f"w2_{f % NH}")
                            nc.sync.dma_start(
                                out=w2t[:], in_=w2_d[128 * f:128 * (f + 1), :])
                            w2_t.append(w2t)
                        for e in range(NE):
                            for f in range(NH, NF):
                                nc.tensor.matmul(ff_ps[e][:],
                                                 w2_t[f][:, 128 * e:128 * (e + 1)],
                                                 r_t[f][:],
                                                 start=False, stop=(f == NF - 1))
                            o = workp.tile([128, ROWS], FP32, name=f"o{e}",
                                           tag="o", bufs=3)
                            nc.vector.scalar_tensor_tensor(
                                o[:], ff_ps[e][:], vecs["b2"][:, e:e + 1],
                                x_t[e][:], OP.add, OP.add)
                            nc.sync.dma_start(out=out_d[128 * e:128 * (e + 1), :],
                                              in_=o[:])

    nc.compile()
    return nc


def _shard(inputs):
    bf = ml_dtypes.bfloat16
    f8 = ml_dtypes.float8_e4m3
    data = np.asarray(inputs["data"], np.float32)
    Wq = np.asarray(inputs["Wq"], np.float32)
    Wk = np.asarray(inputs["Wk"], np.float32)
    Wv = np.asarray(inputs["Wv"], np.float32)

    def to_f8(x):
        x = np.asarray(x, np.float32)
        assert np.abs(x).max() < 235.0, "fp8 overflow risk"
        return np.ascontiguousarray(x.astype(f8))

    wfc = to_f8(np.asarray(inputs["Wfc"], np.float32) * WS)
    W1 = np.asarray(inputs["W1"], np.float32)
    # w1l[128f+p, 128a+c] = W1[128a+p, 128f+c] so each [128,1024] f-tile is a
    # contiguous DMA
    w1l = np.ascontiguousarray(
        W1.reshape(NE, 128, NF, 128).transpose(2, 1, 0, 3)
        .reshape(F4, E).astype(bf))
    w2 = np.ascontiguousarray(np.asarray(inputs["W2"], np.float32).astype(bf))
    kk, qq = np.meshgrid(np.arange(128), np.arange(128), indexing="ij")
    mask = np.ascontiguousarray((kk <= qq).astype(bf))
    common = dict(wfc=wfc, w1l=w1l, w2=w2, mask=mask)
    common["wq"] = to_f8(Wq.transpose(1, 0, 2).reshape(E, H * D) * WS)
    common["wk"] = to_f8(Wk.transpose(1, 0, 2).reshape(E, H * D) * WS)
    common["wv"] = to_f8(Wv.transpose(1, 0, 2).reshape(E, H * D) * WS)
    for nm in ("g1", "be1", "g2", "be2", "bfc", "b1", "b2"):
        common[nm] = np.ascontiguousarray(np.asarray(inputs[nm], np.float32))
    in_maps = []
    for c in range(NC):
        rows = np.concatenate([data[0, RPB * c:RPB * (c + 1)],
                               data[1, RPB * c:RPB * (c + 1)]], axis=0)  # [512, E]
        m = dict(common)
        m["dataT"] = np.ascontiguousarray(rows.T)
        in_maps.append(m)
    return in_maps


_nc_cache = None


def kernel(**inputs):
    global _last_result, _nc_cache
    if _nc_cache is None:
        _nc_cache = _build()
    in_maps = _shard(inputs)
    res = bass_utils.run_bass_kernel_spmd(
        _nc_cache, in_maps, core_ids=list(range(NC)))
    _last_result = res
    out = np.zeros((B, T, E), np.float32)
    for c in range(NC):
        ot = np.asarray(res.results[c]["outT"], np.float32)  # [E, 512]
        out[0, RPB * c:RPB * (c + 1)] = ot[:, 0:RPB].T
        out[1, RPB * c:RPB * (c + 1)] = ot[:, RPB:ROWS].T
    return out


# revision 27
# speedup vs baseline: 1.0770x; 1.0770x over previous
"""Distributed Trainium2 (8 NeuronCores) kernel for a pre-LN transformer block.

Reference computation (B=2, T=2048, E=1024, H=16, D=64):
    h1 = LN(data); q,k,v = per-head projections; causal attention (scale E^-0.5);
    x = data + concat @ Wfc + bfc; out = x + relu(LN(x) @ W1 + b1) @ W2 + b2

Sharding (Ulysses-style, SPMD-uniform across the 8 cores):
  - rows (b,t) are sharded: core c owns rows [256c, 256c+256) of each batch
    (512 rows/core, held transposed as [E, 512], col order [b0|b1])
  - LN1 + all-head QKV projections computed on local rows in fp8 DoubleRow
    (weights host-prescaled x32; scales folded into the softmax exp and the
    Wfc epilogue), then three combined-batch AllToAlls (Q/V/K) move to head
    sharding; a tiny AllToAll fires at kernel start to absorb collective
    warmup/launch skew off the critical path
  - heads sharded: core c owns heads {2c, 2c+1}; full-T causal attention;
    scores stay transposed [keys, q]; 2 heads pack the 64-deep contraction
    via tile_position row groups; softmax denominators come free from a
    ones-column in V; exp probabilities are written fp8 and the P@V matmul
    runs fp8 DoubleRow over key-chunk pairs
  - per-batch AllToAlls return attention output (fp8, x32) to row sharding
  - Wfc (fp8 DoubleRow) + residual + LN2 + FFN (bf16: fp8 fails the accuracy
    budget here) + residual computed on local rows, pipelined by batch half;
    W2 accumulates e-major with all weights prefetched so outputs drain
    progressively instead of in a serial tail
All collective payloads are fp8. LN stats and softmax exp in f32.
"""
import numpy as np
import ml_dtypes

import concourse.bass as bass
import concourse.bacc as bacc
import concourse.tile as tile
from concourse import mybir
from concourse import bass_utils

FP32 = mybir.dt.float32
BF16 = mybir.dt.bfloat16
FP8 = mybir.dt.float8e4
AF = mybir.ActivationFunctionType
OP = mybir.AluOpType
DR = mybir.MatmulPerfMode.DoubleRow

B, T, E, H, D = 2, 2048, 1024, 16, 64
NC = 8
RPB = T // NC            # 256 rows per batch per core
ROWS = B * RPB           # 512 local rows
NE = E // 128            # 8 tiles over E
NEP = NE // 2            # 4 DoubleRow pairs over E
F4 = 4 * E
NF = F4 // 128           # 32 tiles over 4E
NKT = T // 128           # 16 key tiles per batch
EPS = 1e-5
WS = 32.0                # host prescale on Wq/Wk/Wv/Wfc (folded back downstream)
SC2 = (float(E) ** -0.5) / (WS * WS)   # exp scale with q,k both carrying x32
RG = [list(range(NC))]
USE_DR = True

_last_result = None  # BassKernelResults from the most recent run (for test harness)


def _layernorm(nc, tc, workp, statsp, eps1, x_tiles, g_col, b_col, out_factory,
               psname, c0=0, ncols=ROWS):
    """LayerNorm over the E (partition) axis of 8 [128, *] f32 tiles,
    restricted to columns [c0, c0+ncols). bf16 column sums on PE; f32 stats;
    per-row scale/shift broadcast via K=1 matmuls."""
    ones128 = workp.tile([128, 1], BF16, name=f"{psname}_ones128",
                         tag="lno", bufs=2)
    nc.vector.memset(ones128[:], 1.0)
    cs = slice(c0, c0 + ncols)

    with tc.tile_pool(name=psname, bufs=1, space="PSUM") as ps:
        sum_ps = ps.tile([1, ncols], FP32, name=f"{psname}_sum", tag="sum")
        ssq_ps = ps.tile([1, ncols], FP32, name=f"{psname}_ssq", tag="ssq")
        for e in range(NE):
            xb = workp.tile([128, ncols], BF16, name=f"{psname}_xb{e}",
                            tag="lnsrc", bufs=2)
            nc.vector.tensor_copy(xb[:], x_tiles[e][:, cs])
            sq = workp.tile([128, ncols], BF16, name=f"{psname}_sq{e}",
                            tag="lnsq", bufs=2)
            nc.scalar.activation(sq[:], x_tiles[e][:, cs], AF.Square)
            nc.tensor.matmul(sum_ps[:], ones128[:], xb[:],
                             start=(e == 0), stop=(e == NE - 1))
            nc.tensor.matmul(ssq_ps[:], ones128[:], sq[:],
                             start=(e == 0), stop=(e == NE - 1))
        mean = statsp.tile([1, ncols], FP32, name=f"{psname}_mean", tag="v0")
        nc.vector.tensor_scalar_mul(mean[:], sum_ps[:], 1.0 / E)
        msq = statsp.tile([1, ncols], FP32, name=f"{psname}_msq", tag="v1")
        nc.vector.tensor_mul(msq[:], mean[:], mean[:])
        var = statsp.tile([1, ncols], FP32, name=f"{psname}_var", tag="v2")
        nc.vector.scalar_tensor_tensor(var[:], ssq_ps[:], 1.0 / E, msq[:],
                                       OP.mult, OP.subtract)
        std = statsp.tile([1, ncols], FP32, name=f"{psname}_std", tag="v3")
        nc.scalar.activation(std[:], var[:], AF.Sqrt, bias=eps1[:, 0:1])
        rstd = statsp.tile([1, ncols], FP32, name=f"{psname}_rstd", tag="v4")
        nc.vector.reciprocal(rstd[:], std[:])
        nmrn = statsp.tile([1, ncols], FP32, name=f"{psname}_nmrn", tag="v5")
        nc.vector.scalar_tensor_tensor(nmrn[:], mean[:], -1.0, rstd[:],
                                       OP.mult, OP.mult)
        # broadcast rstd / -mean*rstd across partitions on the (idle) GpSimd
        # engine: keeps them out of PSUM, so the next phase's matmul banks
        # don't WAR-wait on the LN epilogue
        bA = workp.tile([128, ncols], FP32, name=f"{psname}_bA",
                        tag="lnbA", bufs=2)
        nc.gpsimd.partition_broadcast(bA[:], rstd[:])
        bB = workp.tile([128, ncols], FP32, name=f"{psname}_bB",
                        tag="lnbB", bufs=2)
        nc.gpsimd.partition_broadcast(bB[:], nmrn[:])
        for e in range(NE):
            t1 = workp.tile([128, ncols], FP32, name=f"{psname}_t1_{e}",
                            tag="lnt1", bufs=2)
            nc.vector.tensor_mul(t1[:], x_tiles[e][:, cs], bA[:])
            t2 = workp.tile([128, ncols], FP32, name=f"{psname}_t2_{e}",
                            tag="lnt2", bufs=2)
            nc.vector.tensor_add(t2[:], t1[:], bB[:])
            o = out_factory(e)
            nc.scalar.activation(o[:, cs], t2[:], AF.Identity,
                                 bias=b_col(e), scale=g_col(e))


def _build():
    nc = bacc.Bacc("TRN2", target_bir_lowering=False, debug=False, num_devices=NC)

    dataT_d = nc.dram_tensor("dataT", [E, ROWS], FP32, kind="ExternalInput")
    wq_d = nc.dram_tensor("wq", [E, H * D], FP8, kind="ExternalInput")
    wk_d = nc.dram_tensor("wk", [E, H * D], FP8, kind="ExternalInput")
    wv_d = nc.dram_tensor("wv", [E, H * D], FP8, kind="ExternalInput")
    wfc_d = nc.dram_tensor("wfc", [H * D, E], FP8, kind="ExternalInput")
    w1_d = nc.dram_tensor("w1l", [F4, E], BF16, kind="ExternalInput")
    w2_d = nc.dram_tensor("w2", [F4, E], BF16, kind="ExternalInput")
    mask_d = nc.dram_tensor("mask", [128, 128], BF16, kind="ExternalInput")
    g1_d = nc.dram_tensor("g1", [E], FP32, kind="ExternalInput")
    be1_d = nc.dram_tensor("be1", [E], FP32, kind="ExternalInput")
    g2_d = nc.dram_tensor("g2", [E], FP32, kind="ExternalInput")
    be2_d = nc.dram_tensor("be2", [E], FP32, kind="ExternalInput")
    bfc_d = nc.dram_tensor("bfc", [E], FP32, kind="ExternalInput")
    b1_d = nc.dram_tensor("b1", [F4], FP32, kind="ExternalInput")
    b2_d = nc.dram_tensor("b2", [E], FP32, kind="ExternalInput")
    out_d = nc.dram_tensor("outT", [E, ROWS], FP32, kind="ExternalOutput")

    def mm_pairs(ps, lhs_pair, rhs_pair, c, clast):
        """One DoubleRow (or two fallback) matmuls accumulating pair c."""
        if USE_DR:
            nc.tensor.matmul(ps, lhs_pair, rhs_pair,
                             start=(c == 0), stop=(c == clast),
                             perf_mode=DR)
        else:
            for sl in range(2):
                nc.tensor.matmul(ps, lhs_pair[:, sl], rhs_pair[:, sl],
                                 start=(c == 0 and sl == 0),
                                 stop=(c == clast and sl == 1))

    with tile.TileContext(nc) as tc:
        with (
            tc.tile_pool(name="constp", bufs=1) as constp,
            tc.tile_pool(name="datap", bufs=1) as datap,
            tc.tile_pool(name="workp", bufs=4) as workp,
            tc.tile_pool(name="statsp", bufs=1) as statsp,
            tc.tile_pool(name="xhp", bufs=1) as xhp,
            tc.tile_pool(name="w1p", bufs=1) as w1p,
            tc.tile_pool(name="dramp", bufs=1, space="DRAM") as dramp,
        ):
            # skew/warmup absorber: a tiny collective nothing depends on,
            # fired before any compute so the first real A2A finds the CC
            # ring warm and the cores synced
            dumb_in = dramp.tile([NC * 128, 4], FP8, name="dumb_in", tag="di")
            dumb_out = dramp.tile([NC * 128, 4], FP8, name="dumb_out", tag="do")
            zed = constp.tile([128, NC * 4], FP8, name="zed", tag="zed")
            nc.vector.memset(zed[:], 0.0)
            nc.sync.dma_start(
                out=dumb_in[:, :].rearrange("(s p) x -> p s x", p=128),
                in_=zed[:, :].rearrange("p (s x) -> p s x", s=NC))
            nc.gpsimd.collective_compute(
                "AllToAll", OP.bypass, replica_groups=RG,
                ins=[dumb_in[:, :].opt()], outs=[dumb_out[:, :].opt()])

            # ---------- constant / input loads ----------
            mask_sb = constp.tile([128, 128], BF16, name="mask_sb", tag="mask")
            nc.sync.dma_start(out=mask_sb[:], in_=mask_d[:, :])
            eps1 = constp.tile([1, 1], FP32, name="eps1", tag="eps1")
            nc.vector.memset(eps1[:], EPS)
            # preload the sqrt activation table while the data DMAs stream,
            # keeping the ~2.7us ACT_TABLE_LOAD out of LN1's critical chain
            sqw0 = workp.tile([1, 1], FP32, name="sqw0", tag="warm", bufs=2)
            nc.scalar.activation(sqw0[:], eps1[:, 0:1], AF.Sqrt)
            vecs = {}
            for nm, dd, w in (("g1", g1_d, NE), ("be1", be1_d, NE), ("g2", g2_d, NE),
                              ("be2", be2_d, NE), ("bfc", bfc_d, NE), ("b2", b2_d, NE),
                              ("b1", b1_d, NF)):
                t = constp.tile([128, w], FP32, name=f"{nm}_sb", tag=nm)
                nc.sync.dma_start(out=t[:], in_=dd.ap().rearrange("(a b) -> b a", b=128))
                vecs[nm] = t

            data_t = []
            for e in range(NE):
                dt_ = datap.tile([128, ROWS], FP32, name=f"data{e}", tag=f"data{e}")
                nc.sync.dma_start(out=dt_[:], in_=dataT_d[128 * e:128 * (e + 1), :])
                data_t.append(dt_)

            # DRAM bounce buffers for the collectives (all fp8).
            # Q and K ride one A2A: block ft = [Q 128 | K 128] x 512 rows.
            qb_in = dramp.tile([NC * 128, ROWS], FP8, name="qb_in", tag="qb_in")
            qb_out = dramp.tile([NC * 128, ROWS], FP8, name="qb_out", tag="qb_out")
            kb_in = dramp.tile([NC * 128, ROWS], FP8, name="kb_in", tag="kb_in")
            kb_out = dramp.tile([NC * 128, ROWS], FP8, name="kb_out", tag="kb_out")
            vb_in = [dramp.tile([NC * 256, 128], FP8, name=f"vb_in{b}",
                                tag=f"vb_in{b}") for b in range(B)]
            vb_out = [dramp.tile([NC * 256, 128], FP8, name=f"vb_out{b}",
                                 tag=f"vb_out{b}") for b in range(B)]
            a2a_in = [dramp.tile([NC * 128, RPB], FP8, name=f"a2a_in{b}",
                                 tag=f"a2a_in{b}") for b in range(B)]
            a2a_out = [dramp.tile([NC * 128, RPB], FP8, name=f"a2a_out{b}",
                                  tag=f"a2a_out{b}") for b in range(B)]

            with (
                tc.tile_pool(name="wfcp", bufs=1) as wfcp,
                tc.tile_pool(name="ccp", bufs=1) as ccp,
            ):
                # cc_all[p, s, b*256+x] = concat (x32 fp8) for hd-block s
                cc_all = ccp.tile([128, NE, ROWS], FP8, name="cc_all", tag="cc")
                with (
                    tc.tile_pool(name="qtp", bufs=1) as qtp,
                    tc.tile_pool(name="vp", bufs=1) as vp,
                    tc.tile_pool(name="clp", bufs=1) as clp,
                ):
                    QTb = [qtp.tile([128, T], FP8, name=f"QT{b}", tag=f"QT{b}")
                           for b in range(B)]
                    KTb = [qtp.tile([128, T], FP8, name=f"KT{b}", tag=f"KT{b}")
                           for b in range(B)]
                    # v_ab[p, k, 96*h + x]; x=64 is the ones column
                    v_ab = [vp.tile([128, NKT, 192], FP8, name=f"v_all{b}",
                                    tag=f"v_all{b}") for b in range(B)]
                    for b in range(B):
                        nc.vector.memset(
                            v_ab[b][:, :, :].rearrange(
                                "p k (h x) -> p k h x", h=2)[:, :, :, 64:65],
                            1.0)
                    concatL = clp.tile([128, B * T], FP8, name="concatL",
                                       tag="concatL")

                    # --- LN1, fp8 DoubleRow QKV, combined-batch A2As ---
                    with (
                        tc.tile_pool(name="h1lp", bufs=1) as h1lp,
                        tc.tile_pool(name="wqkvp", bufs=1) as wqkvp,
                        tc.tile_pool(name="qklp", bufs=1) as qklp,
                    ):
                        h1p = [h1lp.tile([128, 2, ROWS], FP8, name=f"h1p{c}",
                                         tag=f"h1p{c}") for c in range(NEP)]
                        wq_t, wk_t, wv_t = [], [], []
                        for nm, dd, lst in (("wq", wq_d, wq_t), ("wk", wk_d, wk_t),
                                            ("wv", wv_d, wv_t)):
                            for c in range(NEP):
                                t = wqkvp.tile([128, 2, H * D], FP8,
                                               name=f"{nm}p{c}", tag=f"wqkv{c}",
                                               bufs=2)
                                nc.sync.dma_start(
                                    out=t[:, :, :],
                                    in_=dd[256 * c:256 * (c + 1), :].rearrange(
                                        "(two p) f -> p two f", two=2))
                                lst.append(t)

                        _layernorm(nc, tc, workp, statsp, eps1, data_t,
                                   lambda e: vecs["g1"][:, e:e + 1],
                                   lambda e: vecs["be1"][:, e:e + 1],
                                   lambda e: h1p[e // 2][:, e % 2, :], "ln1")

                        with tc.tile_pool(name="psqkv", bufs=1,
                                          space="PSUM") as psqkv:
                            # Q then K projections, separate A2As: Q's wire
                            # drain hides under the K chains' compute
                            for nm, wt, bi, bo in (("q", wq_t, qb_in, qb_out),
                                                   ("k", wk_t, kb_in, kb_out)):
                                pss = [psqkv.tile([128, ROWS], FP32,
                                                  name=f"ps{nm}{i}", tag=f"mm{i}",
                                                  bufs=1) for i in range(NE)]
                                for c in range(NEP):
                                    for ft in range(NE):
                                        mm_pairs(pss[ft][:],
                                                 wt[c][:, :, 128 * ft:128 * (ft + 1)],
                                                 h1p[c][:, :, :], c, NEP - 1)
                                for ft in range(NE):
                                    lt = qklp.tile([128, ROWS], FP8,
                                                   name=f"l{nm}{ft}", tag="qklq",
                                                   bufs=3)
                                    nc.vector.tensor_copy(lt[:], pss[ft][:])
                                    nc.sync.dma_start(
                                        out=bi[128 * ft:128 * (ft + 1), :],
                                        in_=lt[:])
                                nc.gpsimd.collective_compute(
                                    "AllToAll", OP.bypass, replica_groups=RG,
                                    ins=[bi[:, :].opt()],
                                    outs=[bo[:, :].opt()])
                            expw = workp.tile([1, 1], FP32, name="expw",
                                              tag="warm", bufs=2)
                            nc.scalar.activation(expw[:], eps1[:, 0:1], AF.Exp)

                            # V projection (rows on partitions), batch-0 chains
                            # first so its A2A fires before batch-1's; each
                            # batch's exchange lands just before its attention
                            pss = [psqkv.tile([128, 512], FP32,
                                              name=f"psv{i}", tag=f"mm{i}",
                                              bufs=1) for i in range(NE)]
                            for bb in range(B):
                                for i in range(NE):
                                    g, rt = divmod(i, 4)
                                    if rt // 2 != bb:
                                        continue
                                    for c in range(NEP):
                                        mm_pairs(pss[i][:],
                                                 h1p[c][:, :, 128 * rt:128 * (rt + 1)],
                                                 wv_t[c][:, :, 512 * g:512 * (g + 1)],
                                                 c, NEP - 1)
                                    jj = rt % 2
                                    lv = qklp.tile([128, 512], FP8,
                                                   name=f"lv{i}", tag="qklq",
                                                   bufs=3)
                                    nc.vector.tensor_copy(lv[:], pss[i][:])
                                    nc.sync.dma_start(
                                        out=vb_in[bb][:, :].rearrange(
                                            "(ft j p) x -> p ft j x",
                                            ft=NE, j=2)[:, 4 * g:4 * g + 4, jj, :],
                                        in_=lv[:, :].rearrange(
                                            "p (f x) -> p f x", f=4))
                                nc.gpsimd.collective_compute(
                                    "AllToAll", OP.bypass, replica_groups=RG,
                                    ins=[vb_in[bb][:, :].opt()],
                                    outs=[vb_out[bb][:, :].opt()])

                        # readbacks, one DMA each, in collective order Q, K, V
                        for dst, bo in ((QTb, qb_out), (KTb, kb_out)):
                            for b in range(B):
                                nc.scalar.dma_start(
                                    out=dst[b][:, :].rearrange(
                                        "p (s x) -> p s x", s=NC),
                                    in_=bo[:, :].rearrange(
                                        "(s p) (b x) -> p b s x",
                                        p=128, b=B)[:, b, :, :])
                        for b in range(B):
                            for hi in range(2):
                                nc.scalar.dma_start(
                                    out=v_ab[b][:, :, :].rearrange(
                                        "p k (h x) -> p k h x",
                                        h=2)[:, :, hi, 0:64],
                                    in_=vb_out[b][:, :].rearrange(
                                        "(k p) (h x) -> p k h x",
                                        p=128, h=2)[:, :, hi, :])

                    # prefetch Wfc (fp8 pairs) while attention runs
                    wfc_t = []
                    for s in range(NEP):
                        t = wfcp.tile([128, 2, E], FP8, name=f"wfcp{s}",
                                      tag=f"wfc{s}")
                        nc.sync.dma_start(
                            out=t[:, :, :],
                            in_=wfc_d[256 * s:256 * (s + 1), :].rearrange(
                                "(two p) e -> p two e", two=2))
                        wfc_t.append(t)

                    # ------- causal attention for 2 heads, DR P@V -------
                    with (
                        tc.tile_pool(name="pst", bufs=2, space="PSUM") as pst,
                        tc.tile_pool(name="pot", bufs=4, space="PSUM") as pot,
                    ):
                        for b in range(B):
                            for qc in range(T // 512):
                                q0 = 512 * qc
                                nk = 4 * qc + 4
                                ots = []
                                for hi in range(2):
                                    ots.append(pot.tile([65, 512], FP32,
                                                        name=f"ot{b}_{qc}_{hi}",
                                                        tag="ot"))
                                def emit_av(pk, poff, ppexp, last):
                                    for hi in range(2):
                                        nc.tensor.matmul(
                                            ots[hi][:, poff:512],
                                            v_ab[b][:, pk, 96 * hi:96 * hi + 65],
                                            ppexp[:, 512 * hi + poff:
                                                  512 * hi + 512],
                                            start=(pk == 0), stop=last)

                                # software pipeline: scores(k+1) is emitted
                                # before P@V(k), so the PE fills each chunk's
                                # exp latency with the next chunk's scores
                                pend = None
                                for k in range(nk):
                                    off = max(0, 128 * k - q0)
                                    st = pst.tile([128, 1024], FP32,
                                                  name=f"st{b}_{qc}_{k}",
                                                  tag="st")
                                    pexp = workp.tile(
                                        [128, 1024], FP8,
                                        name=f"pex{b}_{qc}_{k}",
                                        tag="pexp", bufs=4)
                                    for hi in range(2):
                                        hp = slice(64 * hi, 64 * (hi + 1))
                                        nc.tensor.matmul(
                                            st[:, 512 * hi + off:512 * hi + 512],
                                            KTb[b][hp, 128 * k:128 * (k + 1)],
                                            QTb[b][hp, q0 + off:q0 + 512],
                                            start=True, stop=True,
                                            tile_position=(64 * hi, 0))
                                    nc.scalar.activation(
                                        pexp[:, :].rearrange(
                                            "p (h x) -> p h x", h=2)[:, :, off:512],
                                        st[:, :].rearrange(
                                            "p (h x) -> p h x", h=2)[:, :, off:512],
                                        AF.Exp, scale=SC2)
                                    if k >= 4 * qc:  # diagonal: causal mask
                                        for hi in range(2):
                                            nc.vector.tensor_mul(
                                                pexp[:, 512 * hi + off:
                                                     512 * hi + off + 128],
                                                pexp[:, 512 * hi + off:
                                                     512 * hi + off + 128],
                                                mask_sb[:])
                                    if pend is not None:
                                        emit_av(*pend, False)
                                    pend = (k, off, pexp)
                                emit_av(*pend, True)
                                for hi in range(2):
                                    rc = statsp.tile([1, 512], FP32,
                                                     name=f"rc{b}_{qc}_{hi}",
                                                     tag="rc", bufs=2)
                                    nc.vector.reciprocal(rc[:], ots[hi][64:65, :])
                                    rbs = workp.tile([64, 512], FP32,
                                                     name=f"rbs{b}_{qc}_{hi}",
                                                     tag="rbs", bufs=3)
                                    nc.gpsimd.partition_broadcast(rbs[:], rc[:])
                                    nc.vector.tensor_mul(
                                        concatL[64 * hi:64 * (hi + 1),
                                                b * T + q0: b * T + q0 + 512],
                                        ots[hi][0:64, :], rbs[:])
                            # batch-b attention done: AllToAll it back to row
                            # sharding while the next batch computes
                            nc.sync.dma_start(
                                out=a2a_in[b][:, :].rearrange(
                                    "(j p) x -> p j x", p=128),
                                in_=concatL[:, b * T:(b + 1) * T].rearrange(
                                    "p (j x) -> p j x", j=NC))
                            nc.gpsimd.collective_compute(
                                "AllToAll", OP.bypass, replica_groups=RG,
                                ins=[a2a_in[b][:, :].opt()],
                                outs=[a2a_out[b][:, :].opt()])
                            nc.scalar.dma_start(
                                out=cc_all[:, :, RPB * b:RPB * (b + 1)],
                                in_=a2a_out[b][:, :].rearrange(
                                    "(s p) x -> p s x", p=128))

                # ---------- batch-half pipelined tail ----------
                x_t, h2_t = [], []
                for e in range(NE):
                    x_t.append(xhp.tile([128, ROWS], BF16, name=f"x{e}",
                                        tag=f"x{e}"))
                    h2_t.append(xhp.tile([128, ROWS], BF16, name=f"h2_{e}",
                                         tag=f"h2{e}"))

                sqw = workp.tile([1, 1], FP32, name="sqw", tag="warm", bufs=2)
                nc.scalar.activation(sqw[:], eps1[:, 0:1], AF.Sqrt)

                def wfc_half(half, ps_pool):
                    for e in range(NE):
                        ps = ps_pool.tile([128, RPB], FP32,
                                          name=f"psx{half}_{e}", tag="mm")
                        for s in range(NEP):
                            mm_pairs(ps[:],
                                     wfc_t[s][:, :, 128 * e:128 * (e + 1)],
                                     cc_all[:, 2 * s:2 * s + 2,
                                            RPB * half:RPB * (half + 1)],
                                     s, NEP - 1)
                        xw = workp.tile([128, RPB], FP32,
                                        name=f"xw{half}_{e}", tag="xw", bufs=3)
                        nc.scalar.activation(xw[:], ps[:], AF.Identity,
                                             bias=vecs["bfc"][:, e:e + 1],
                                             scale=1.0 / (WS * WS))
                        nc.vector.tensor_add(
                            x_t[e][:, RPB * half:RPB * (half + 1)], xw[:],
                            data_t[e][:, RPB * half:RPB * (half + 1)])

                NSPLIT = 16  # zT f-tiles emitted per-half to cover A2A#1

                with (
                    tc.tile_pool(name="rtp", bufs=1) as rtp,
                    tc.tile_pool(name="w2p", bufs=1) as w2p,
                ):
                    r_t = []
                    for f in range(NF):
                        r_t.append(rtp.tile([128, ROWS], BF16, name=f"r{f}",
                                            tag=f"r{f}"))

                    def w1_load(f, tag="w1f", bufs=4):
                        w1f = w1p.tile([128, E], BF16, name=f"w1f{f}", tag=tag,
                                       bufs=bufs)
                        nc.sync.dma_start(out=w1f[:],
                                          in_=w1_d[128 * f:128 * (f + 1), :])
                        return w1f

                    def zt_chain(f, w1f, psz, c0, ncols):
                        ps = psz.tile([128, ncols], FP32,
                                      name=f"psz{f}_{c0}", tag="mm")
                        for e in range(NE):
                            nc.tensor.matmul(
                                ps[:], w1f[:, 128 * e:128 * (e + 1)],
                                h2_t[e][:, c0:c0 + ncols],
                                start=(e == 0), stop=(e == NE - 1))
                        nc.scalar.activation(r_t[f][:, c0:c0 + ncols], ps[:],
                                             AF.Relu, bias=vecs["b1"][:, f:f + 1])

                    with tc.tile_pool(name="psfc", bufs=2, space="PSUM") as psfc:
                        wfc_half(0, psfc)
                    _layernorm(nc, tc, workp, statsp, eps1, x_t,
                               lambda e: vecs["g2"][:, e:e + 1],
                               lambda e: vecs["be2"][:, e:e + 1],
                               lambda e: h2_t[e], "ln2a",
                               c0=0, ncols=RPB)
                    w1fs = {}
                    w2_t = []
                    with tc.tile_pool(name="psz", bufs=2, space="PSUM") as psz:
                        for f in range(NSPLIT):
                            w1fs[f] = w1_load(f, tag=f"w1k{f}", bufs=1)
                            zt_chain(f, w1fs[f], psz, 0, RPB)
                        # batch-1 catch-up (waits on the concat AllToAll)
                        with tc.tile_pool(name="psfc2", bufs=2,
                                          space="PSUM") as psfc2:
                            wfc_half(1, psfc2)
                        _layernorm(nc, tc, workp, statsp, eps1, x_t,
                                   lambda e: vecs["g2"][:, e:e + 1],
                                   lambda e: vecs["be2"][:, e:e + 1],
                                   lambda e: h2_t[e], "ln2b",
                                   c0=RPB, ncols=RPB)
                        for f in range(NSPLIT):
                            zt_chain(f, w1fs[f], psz, RPB, RPB)
                        for f in range(NSPLIT, NF):
                            w1f = w1_load(f)
                            zt_chain(f, w1f, psz, 0, ROWS)
                        # prefetch first-half W2 tiles; the stream overlaps
                        # the tail of the W1 compute
                        NH = NF // 2
                        for f in range(NH):
                            w2t = w2p.tile([128, E], BF16, name=f"w2t{f}",
                                           tag=f"w2_{f % NH}")
                            nc.sync.dma_start(
                                out=w2t[:], in_=w2_d[128 * f:128 * (f + 1), :])
                            w2_t.append(w2t)
                    # two-pass W2 with carried PSUM accumulation (halves the
                    # resident weight footprint); outputs drain per-e in the
                    # second pass instead of in a serial tail
                    with tc.tile_pool(name="psff", bufs=1, space="PSUM") as psff:
                        ff_ps = [psff.tile([128, ROWS], FP32, name=f"ff{e}",
                                           tag=f"ff{e}") for e in range(NE)]
                        for e in range(NE):
                            for f in range(NH):
                                nc.tensor.matmul(ff_ps[e][:],
                                                 w2_t[f][:, 128 * e:128 * (e + 1)],
                                                 r_t[f][:],
                                                 start=(f == 0), stop=False)
                        for f in range(NH, NF):
                            w2t = w2p.tile([128, E], BF16, name=f"w2t{f}",
                                           tag=f"w2_{f % NH}")
                            nc.sync.dma_start(
                                out=w2t[:], in_=w2_d[128 * f:128 * (f + 1), :])
                            w2_t.append(w2t)
                        for e in range(NE):
                            for f in range(NH, NF):
                                nc.tensor.matmul(ff_ps[e][:],
                                                 w2_t[f][:, 128 * e:128 * (e + 1)],
                                                 r_t[f][:],
                                                 start=False, stop=(f == NF - 1))
                            o = workp.tile([128, ROWS], FP32, name=f"o{e}",
                                           tag="o", bufs=3)
                            nc.vector.scalar_tensor_tensor(
                                o[:], ff_ps[e][:], vecs["b2"][:, e:e + 1],
                                x_t[e][:], OP.add, OP.add)
                            nc.sync.dma_start(out=out_d[128 * e:128 * (e + 1), :],
                                              in_=o[:])

    nc.compile()
    return nc


def _shard(inputs):
    bf = ml_dtypes.bfloat16
    f8 = ml_dtypes.float8_e4m3
    data = np.asarray(inputs["data"], np.float32)
    Wq = np.asarray(inputs["Wq"], np.float32)
    Wk = np.asarray(inputs["Wk"], np.float32)
    Wv = np.asarray(inputs["Wv"], np.float32)

    def to_f8(x):
        x = np.asarray(x, np.float32)
        assert np.abs(x).max() < 235.0, "fp8 overflow risk"
        return np.ascontiguousarray(x.astype(f8))

    wfc = to_f8(np.asarray(inputs["Wfc"], np.float32) * WS)
    W1 = np.asarray(inputs["W1"], np.float32)
    # w1l[128f+p, 128a+c] = W1[128a+p, 128f+c] so each [128,1024] f-tile is a
    # contiguous DMA
    w1l = np.ascontiguousarray(
        W1.reshape(NE, 128, NF, 128).transpose(2, 1, 0, 3)
        .reshape(F4, E).astype(bf))
    w2 = np.ascontiguousarray(np.asarray(inputs["W2"], np.float32).astype(bf))
    kk, qq = np.meshgrid(np.arange(128), np.arange(128), indexing="ij")
    mask = np.ascontiguousarray((kk <= qq).astype(bf))
    common = dict(wfc=wfc, w1l=w1l, w2=w2, mask=mask)
    common["wq"] = to_f8(Wq.transpose(1, 0, 2).reshape(E, H * D) * WS)
    common["wk"] = to_f8(Wk.transpose(1, 0, 2).reshape(E, H * D) * WS)
    common["wv"] = to_f8(Wv.transpose(1, 0, 2).reshape(E, H * D) * WS)
    for nm in ("g1", "be1", "g2", "be2", "bfc", "b1", "b2"):
        common[nm] = np.ascontiguousarray(np.asarray(inputs[nm], np.float32))
    in_maps = []
    for c in range(NC):
        rows = np.concatenate([data[0, RPB * c:RPB * (c + 1)],
                               data[1, RPB * c:RPB * (c + 1)]], axis=0)  # [512, E]
        m = dict(common)
        m["dataT"] = np.ascontiguousarray(rows.T)
        in_maps.append(m)
    return in_maps


_nc_cache = None


def kernel(**inputs):
    global _last_result, _nc_cache
    if _nc_cache is None:
        _nc_cache = _build()
    in_maps = _shard(inputs)
    res = bass_utils.run_bass_kernel_spmd(
        _nc_cache, in_maps, core_ids=list(range(NC)))
    _last_result = res
    out = np.zeros((B, T, E), np.float32)
    for c in range(NC):
        ot = np.asarray(res.results[c]["outT"], np.float32)  # [E, 512]
        out[0, RPB * c:RPB * (c + 1)] = ot[:, 0:RPB].T
        out[1, RPB * c:RPB * (c + 1)] = ot[:, RPB:ROWS].T
    return out
